# revision 29
# baseline (speedup 1.0000x reference)
"""MultiHeadDualAttention Trainium2 kernel — single-exp fp8 design (r7).

Sharding: 8 heads -> 8 cores (tensor parallel over heads).

Math per head (exact vs reference modulo fp):
  With FULLY-biased projections k1p = k1@wk1 + b1, k2p = k2@wk2 + b2 and the
  single unnormalized matrix A = exp(SCALE * k1p @ k2p^T):
    o2 = diag(1/rowsum(A)) A v2p      (rowsoftmax is row-scale invariant)
    o1 = A^T v1p / colsum(A)          (colsoftmax is col-scale invariant)
  so ONE exp pass serves both directions (the baseline computed exp twice).
  v-bias commutes through softmax; host adds (bv@wo + bo) constants.

Perf structure (per core):
  - k-projections stored fp8e4 [128, N], d duplicated on both partition halves
    (host ships wk column-duplicated). K=64 score runs as [64,128]x[64,512]
    fp8 matmuls on PE row-halves 0/64 (128-col weights keep FWL on).
  - E is written ONCE as fp8e4 [128, 32(t), 1024] blocks over mu (= permuted
    m). exp splits per pair: A-tile on Scalar (table exp), B-tile on DVE via
    the Schraudolph int8 bit-trick for pairs < ACT_B (i8 = trunc(S*8/ln2*SCALE
    + B8) bitcast fp8e4).
  - E^T via XBAR DMA transpose (16x128 uint16 tiles) on byte-PAIRED fp8,
    issued from the Activation queue right after the exps it depends on:
    mu is host-permuted so uint16 byte pairs = m-tile pairs, lining up with
    DoubleRow's two k-tiles and the natural [128, 32, 80] v-aug layout.
  - PVs are fp8 DoubleRow (2 k-tiles/instr): o1 streams E natively (contract
    n, ifmap [p, 2, 512]), o2 streams ET byte pairs (contract m, [p, 2, 128]).
    Ones-column in v-aug yields the softmax denominators.
  - o2 partials accumulate into SBUF fp32 (DVE adds), DMA'd out per slice.
  - Outputs ship small: o1T [64, N] bf16 + den1, o2acc [65, N] f32. The final
    64x256 wo projections + divisions run on HOST (2% of FLOPs), like the
    baseline's host-side division/bias handling.
  - Per-block emission interleaves score(b) with PV(b-1) chunk-by-chunk so the
    in-order PE stream fills exp-latency gaps.
Host: permutes k2 columns into mu order, unscrambles o1 rows, projects with
wo, sums the 8 head partials, adds bias constants.
"""

import sys

sys.path.insert(0, "/opt/trn_rl_repo")

import numpy as np

N = 4096
C = 256
D = 64
SCALE = float(D) ** -0.5
NCORES = 8
MBLK = 1024          # mu-block width
NMB = N // MBLK      # 4 blocks
NT = 32              # n-tiles of 128
NQ = NT // 2         # t-pairs per block
ACT_B = 10           # B-side exp of pairs 0..ACT_B-1 goes to DVE
A8 = 11.5415603 * SCALE   # 8/ln2 * SCALE (DVE reads raw psum scores)
B8 = 56.1                 # Schraudolph offset: 55.6 tuned for round-to-nearest
                          # +0.5 because the fp32->int8 convert truncates
                          # (values are all positive, so trunc == floor)
TP_ENGINE = "sync"        # queue for the XBAR transposes: "scalar" | "sync"

_cache: dict = {}


def _build_module():
    import concourse.bacc as bacc
    import concourse.mybir as mybir
    import concourse.tile as tile

    f32 = mybir.dt.float32
    bf16 = mybir.dt.bfloat16
    f8 = mybir.dt.float8e4
    u16 = mybir.dt.uint16
    i8 = mybir.dt.int8
    Exp = mybir.ActivationFunctionType.Exp
    Copy = mybir.ActivationFunctionType.Copy
    DR = mybir.MatmulPerfMode.DoubleRow
    Add = mybir.AluOpType.add
    Mult = mybir.AluOpType.mult

    nc = bacc.Bacc("TRN2", target_bir_lowering=False, debug=False)
    tp_eng = nc.scalar if TP_ENGINE == "scalar" else nc.sync

    def din(name, shape, dt=bf16):
        return nc.dram_tensor(name, shape, dt, kind="ExternalInput").ap()

    k1T = din("k1T", [C, N])      # natural n columns
    k2T = din("k2T", [C, N])      # mu-PERMUTED columns (host)
    v1T = din("v1T", [C, N])
    v2T = din("v2T", [C, N])
    wk1 = din("wk1", [C, 128])    # column-duplicated [wk|wk]
    wk2 = din("wk2", [C, 128])
    wv1 = din("wv1", [C, D])
    wv2 = din("wv2", [C, D])
    bk1 = din("bk1", [128, 1], f32)   # row-duplicated bias
    bk2 = din("bk2", [128, 1], f32)

    o1b = nc.dram_tensor("o1b", [D, N], bf16, kind="ExternalOutput").ap()
    den1 = nc.dram_tensor("den1", [1, N], f32, kind="ExternalOutput").ap()
    o2a = nc.dram_tensor("o2a", [D + 1, N], f32, kind="ExternalOutput").ap()

    with tile.TileContext(nc) as tc:
        with (
            tc.tile_pool(name="const", bufs=1) as constp,
            tc.tile_pool(name="raw", bufs=4) as rawp,
            tc.tile_pool(name="eblk", bufs=2) as ep,
            tc.tile_pool(name="et", bufs=2) as etp,
            tc.tile_pool(name="o1t", bufs=2) as otp,
            tc.tile_pool(name="denp", bufs=2) as denp,
            tc.tile_pool(name="sps", bufs=2, space="PSUM") as sps,
            tc.tile_pool(name="p1", bufs=1, space="PSUM") as p1,
            tc.tile_pool(name="p2", bufs=2, space="PSUM") as p2,
        ):
            # ---- PE warm-up (HAM clock ramp) ----
            warm = constp.tile([128, 512], bf16, tag="warm")
            nc.vector.memset(warm[:], 0.0)
            for _ in range(14):
                wps = p2.tile([128, 512], f32, tag="op")
                nc.tensor.matmul(wps[:], warm[:, 0:128], warm[:], start=True, stop=True)

            # earliest input prefetch: the raws block 0's kproj needs
            pre = []
            for rawT, j in ((k2T, 0), (k2T, 1), (k1T, 0)):
                raw = rawp.tile([128, 2, 512], bf16, tag="raw")
                for ct in range(2):
                    nc.sync.dma_start(
                        out=raw[:, ct, :],
                        in_=rawT[ct * 128:(ct + 1) * 128, j * 512:(j + 1) * 512])
                pre.append(raw)

            # ---- weights ----
            def ld(name, drt, shape):
                t = constp.tile(shape, bf16, tag=name)
                for ct in range(2):
                    nc.sync.dma_start(out=t[:, ct, :], in_=drt[ct * 128:(ct + 1) * 128, :])
                return t

            wk1_sb = ld("wk1", wk1, [128, 2, 128])
            wk2_sb = ld("wk2", wk2, [128, 2, 128])
            wv1_sb = ld("wv1", wv1, [128, 2, D])
            wv2_sb = ld("wv2", wv2, [128, 2, D])
            bk1_sb = constp.tile([128, 1], f32, tag="bk1")
            nc.sync.dma_start(out=bk1_sb[:], in_=bk1[:])
            bk2_sb = constp.tile([128, 1], f32, tag="bk2")
            nc.sync.dma_start(out=bk2_sb[:], in_=bk2[:])

            # o2 accumulator [65, N] fp32 (row 64 = denominator)
            o2acc = constp.tile([D + 1, N], f32, tag="o2acc")
            nc.vector.memset(o2acc[:], 0.0)

            k1p8 = constp.tile([128, N], f8, tag="k1p8")
            k2p8 = constp.tile([128, N], f8, tag="k2p8")
            vaug1 = constp.tile([128, NT, 80], f8, tag="vaug1")
            vaug2 = constp.tile([128, NT, 80], f8, tag="vaug2")
            nc.vector.memset(vaug1[:, :, D:D + 1], 1.0)
            nc.vector.memset(vaug2[:, :, D:D + 1], 1.0)

            # one k-projection chunk -> k?p8[:, 512j:512(j+1)] (d duplicated)
            def k_chunk(rawT, wsb, bsb, dest, j, raw=None):
                if raw is None:
                    raw = rawp.tile([128, 2, 512], bf16, tag="raw")
                    for ct in range(2):
                        nc.sync.dma_start(
                            out=raw[:, ct, :],
                            in_=rawT[ct * 128:(ct + 1) * 128, j * 512:(j + 1) * 512])
                kp = p2.tile([128, 512], f32, tag="op")
                for ct in range(2):
                    nc.tensor.matmul(kp[:], wsb[:, ct, :], raw[:, ct, :],
                                     start=(ct == 0), stop=(ct == 1))
                nc.vector.tensor_scalar(
                    dest[:, j * 512:(j + 1) * 512], kp[:], bsb[:], None, Add)

            # one v-projection chunk -> vaug[:, 4j:4j+4, 0:64]
            def v_chunk(rawT, wsb, dest, j):
                raw = rawp.tile([128, 2, 512], bf16, tag="raw")
                for ct in range(2):
                    nc.sync.dma_start(
                        out=raw[:, ct, :],
                        in_=rawT[ct * 128:(ct + 1) * 128, j * 512:(j + 1) * 512])
                vp = p2.tile([128, 4, D], f32, tag="op")
                for k in range(4):
                    for ct in range(2):
                        nc.tensor.matmul(
                            vp[:, k, :], raw[:, ct, k * 128:(k + 1) * 128],
                            wsb[:, ct, :], start=(ct == 0), stop=(ct == 1))
                nc.vector.tensor_copy(dest[:, 4 * j:4 * j + 4, 0:D], vp[:])

            # minimal prologue: the chunks score block 0 needs first
            k_chunk(k2T, wk2_sb, bk2_sb, k2p8, 0, raw=pre[0])
            k_chunk(k2T, wk2_sb, bk2_sb, k2p8, 1, raw=pre[1])
            k_chunk(k1T, wk1_sb, bk1_sb, k1p8, 0, raw=pre[2])
            # remaining projection work, early in the PE stream; block-0
            # exp-throttling gives PE slack to absorb it
            for j in range(2, 8):
                k_chunk(k2T, wk2_sb, bk2_sb, k2p8, j)
            for j in range(1, 8):
                k_chunk(k1T, wk1_sb, bk1_sb, k1p8, j)
            for j in range(8):
                v_chunk(v1T, wv1_sb, vaug1, j)
            for j in range(8):
                v_chunk(v2T, wv2_sb, vaug2, j)

            state = {}

            def emit_o1_chunk(q, pEhs, o1ps):
                pE = pEhs[q // 8]
                t0 = (2 * q) % (NT // 2)
                for c in range(2):
                    nc.tensor.matmul(
                        o1ps[:, c * 512:(c + 1) * 512],
                        vaug1[:, 2 * q:2 * q + 2, 0:D + 1],
                        pE[:, t0:t0 + 2, c * 512:(c + 1) * 512],
                        perf_mode=DR, start=(q == 0), stop=(q == NQ - 1))

            def emit_o2_chunk(q, pb, pEThs, stream_o2=False):
                n8 = q // 2
                if q % 2 == 0:
                    state[f"o2ps{pb}"] = p2.tile(
                        [D + 1, 512], f32, tag="op", name=f"o2ps_{pb}_{q}")
                o2ps = state[f"o2ps{pb}"]
                pET8 = pEThs[q // 8].bitcast(f8)
                for nl in (0, 1) if q % 2 == 0 else (2, 3):
                    t = (4 * n8 + nl) % (NT // 2)
                    for c2 in range(4):
                        ifm = pET8[:, t, c2, :].rearrange("p (l i) -> p i l", i=2)
                        nc.tensor.matmul(
                            o2ps[:, nl * 128:(nl + 1) * 128],
                            vaug2[:, 8 * pb + 2 * c2:8 * pb + 2 * c2 + 2, 0:D + 1],
                            ifm, perf_mode=DR, start=(c2 == 0), stop=(c2 == 3))
                if q % 2 == 1:
                    nc.vector.tensor_tensor(
                        o2acc[:, n8 * 512:(n8 + 1) * 512],
                        o2ps[:], o2acc[:, n8 * 512:(n8 + 1) * 512], Add)
                    if stream_o2:
                        nc.sync.dma_start(
                            out=o2a[:, n8 * 512:(n8 + 1) * 512],
                            in_=o2acc[:, n8 * 512:(n8 + 1) * 512])

            def emit_pv_chunk(q, pb, pEhs, pEThs, o1ps, stream_o2=False):
                emit_o1_chunk(q, pEhs, o1ps)
                emit_o2_chunk(q, pb, pEThs, stream_o2)

            def emit_o1_epilogue(pb, o1ps):
                o1t = otp.tile([D, MBLK], bf16, tag="o1t")
                d1 = denp.tile([1, MBLK], f32, tag="d1")
                nc.scalar.activation(o1t[:], o1ps[0:D, :], Copy)
                nc.vector.tensor_copy(d1[:], o1ps[D:D + 1, :])
                nc.sync.dma_start(out=o1b[:, pb * MBLK:(pb + 1) * MBLK], in_=o1t[:])
                nc.sync.dma_start(out=den1[0:1, pb * MBLK:(pb + 1) * MBLK], in_=d1[:])

            def emit_block(mb):
                """score+exp+transpose for block mb, interleaved per t-pair
                with the PV of the previous block."""
                Ehs, EThs = [], []
                prev = state.get("prev")
                if prev is not None:
                    pb, pEhs, pEThs = prev
                    o1ps = p1.tile([D + 1, MBLK], f32, tag="o1",
                                   name=f"o1ps_{mb}")
                for q in range(NQ):
                    if q % 8 == 0:
                        Ehs.append(ep.tile([128, NT // 2, MBLK], f8,
                                           tag=f"eblk{q // 8}",
                                           name=f"E_{mb}_{q // 8}"))
                        EThs.append(etp.tile([128, NT // 2, 4, 128], u16,
                                             tag=f"et{q // 8}",
                                             name=f"ET_{mb}_{q // 8}"))
                    E, ET = Ehs[q // 8], EThs[q // 8]
                    E8i = E.bitcast(i8)
                    Eu16 = E.bitcast(u16)
                    t0, t1 = (2 * q) % NT, (2 * q + 1) % NT
                    spA = sps.tile([128, MBLK], f32, tag="sp")
                    spB = sps.tile([128, MBLK], f32, tag="sp")
                    for c in range(2):
                        lo = mb * MBLK + c * 512
                        nc.tensor.matmul(
                            spA[:, c * 512:(c + 1) * 512],
                            k1p8[0:64, t0 * 128:(t0 + 1) * 128],
                            k2p8[0:64, lo:lo + 512], start=True, stop=True)
                        nc.tensor.matmul(
                            spB[:, c * 512:(c + 1) * 512],
                            k1p8[64:128, t1 * 128:(t1 + 1) * 128],
                            k2p8[64:128, lo:lo + 512], start=True, stop=True)
                    tl0 = (2 * q) % (NT // 2)
                    act_b = 6 if mb == 0 else ACT_B
                    nc.scalar.activation(E[:, tl0, :], spA[:], Exp, scale=SCALE)
                    # PV of the previous block comes BEFORE the B-side exp in
                    # the DVE stream so the o2acc add releases its psum slot
                    # promptly
                    if prev is not None:
                        emit_pv_chunk(q, pb, pEhs, pEThs, o1ps)
                    if mb == NMB - 1 and q >= 2:
                        # interleave the last block's own o2-PV at lag 2
                        emit_o2_chunk(q - 2, mb, EThs, stream_o2=True)
                    if q < act_b:
                        nc.vector.tensor_scalar(
                            E8i[:, tl0 + 1, :], spB[:], A8, B8, Mult, Add)
                    else:
                        nc.scalar.activation(E[:, tl0 + 1, :], spB[:], Exp, scale=SCALE)
                    if q % 2 == 1:
                        tp_eng.dma_start_transpose(
                            ET[:, tl0 - 2:tl0 + 2], Eu16[:, tl0 - 2:tl0 + 2])
                if prev is not None:
                    emit_o1_epilogue(pb, o1ps)
                state["prev"] = (mb, Ehs, EThs)

            for mb in range(NMB):
                emit_block(mb)
            # tail: the last block's o1-PV + final o2 chunks
            pb, pEhs, pEThs = state["prev"]
            o1ps = p1.tile([D + 1, MBLK], f32, tag="o1", name="o1ps_tail")
            for q in range(NQ):
                emit_o1_chunk(q, pEhs, o1ps)
                if q in (NQ - 2, NQ - 1):
                    emit_o2_chunk(q, pb, pEThs, stream_o2=True)
            emit_o1_epilogue(pb, o1ps)

    nc.compile()
    return nc


def _mu_perm():
    """mu -> m mapping: mu = 1024*mb + 256*c2 + 2*q + i  ->
    m = 1024*mb + 128*(2*c2 + i) + q."""
    mu = np.arange(N)
    mb = mu // MBLK
    r = mu % MBLK
    c2 = r // 256
    q = (r % 256) // 2
    i = mu % 2
    return (MBLK * mb + 128 * (2 * c2 + i) + q).astype(np.int64)


def _get_nc():
    if "nc" not in _cache:
        _cache["nc"] = _build_module()
    return _cache["nc"]


def kernel(k1, v1, k2, v2,
           wk1_w, wk1_b, wv1_w, wv1_b,
           wk2_w, wk2_b, wv2_w, wv2_b,
           wo1_w, wo1_b, wo2_w, wo2_b):
    import ml_dtypes
    from concourse.bass_utils import run_bass_kernel_spmd

    nc = _get_nc()

    f = np.float32
    bf = ml_dtypes.bfloat16
    perm = _mu_perm()

    k1T = np.ascontiguousarray(np.asarray(k1, f).T).astype(bf)
    v1T = np.ascontiguousarray(np.asarray(v1, f).T).astype(bf)
    k2T = np.ascontiguousarray(np.asarray(k2, f).T[:, perm]).astype(bf)
    v2T = np.ascontiguousarray(np.asarray(v2, f).T).astype(bf)

    def dup2(a):   # [C, 64] -> [C, 128] column-duplicated
        return np.ascontiguousarray(np.concatenate([a, a], axis=1))

    in_maps = []
    for h in range(NCORES):
        sl = slice(h * D, (h + 1) * D)
        in_maps.append({
            "k1T": k1T, "v1T": v1T, "k2T": k2T, "v2T": v2T,
            "wk1": dup2(np.asarray(wk1_w, f)[:, sl]).astype(bf),
            "wk2": dup2(np.asarray(wk2_w, f)[:, sl]).astype(bf),
            "wv1": np.ascontiguousarray(np.asarray(wv1_w, f)[:, sl]).astype(bf),
            "wv2": np.ascontiguousarray(np.asarray(wv2_w, f)[:, sl]).astype(bf),
            "bk1": np.ascontiguousarray(
                np.tile(np.asarray(wk1_b, f)[sl].reshape(D, 1), (2, 1))),
            "bk2": np.ascontiguousarray(
                np.tile(np.asarray(wk2_b, f)[sl].reshape(D, 1), (2, 1))),
        })

    res = run_bass_kernel_spmd(nc, in_maps, list(range(NCORES)))
    _cache["last_result"] = res

    out1 = np.zeros((N, C), f)
    out2 = np.zeros((N, C), f)
    for h in range(NCORES):
        sl = slice(h * D, (h + 1) * D)
        rh = res.results[h]
        t1 = (np.asarray(rh["o1b"], f) / rh["den1"]).T @ np.asarray(wo1_w, f)[sl, :]
        out1[perm] += t1                     # unscramble mu -> m
        o2acc = np.asarray(rh["o2a"], f)
        out2 += (o2acc[0:D, :] / o2acc[D:D + 1, :]).T @ np.asarray(wo2_w, f)[sl, :]
    out1 += np.asarray(wv1_b, f) @ np.asarray(wo1_w, f) + np.asarray(wo1_b, f)
    out2 += np.asarray(wv2_b, f) @ np.asarray(wo2_w, f) + np.asarray(wo2_b, f)
    return out1, out2


# revision 30
# speedup vs baseline: 1.0119x; 1.0119x over previous
"""MultiHeadDualAttention Trainium2 kernel — single-exp fp8 design (r7).

Sharding: 8 heads -> 8 cores (tensor parallel over heads).

Math per head (exact vs reference modulo fp):
  With FULLY-biased projections k1p = k1@wk1 + b1, k2p = k2@wk2 + b2 and the
  single unnormalized matrix A = exp(SCALE * k1p @ k2p^T):
    o2 = diag(1/rowsum(A)) A v2p      (rowsoftmax is row-scale invariant)
    o1 = A^T v1p / colsum(A)          (colsoftmax is col-scale invariant)
  so ONE exp pass serves both directions (the baseline computed exp twice).
  v-bias commutes through softmax; host adds (bv@wo + bo) constants.

Perf structure (per core):
  - k-projections stored fp8e4 [128, N], d duplicated on both partition halves
    (host ships wk column-duplicated). K=64 score runs as [64,128]x[64,512]
    fp8 matmuls on PE row-halves 0/64 (128-col weights keep FWL on).
  - E is written ONCE as fp8e4 [128, 32(t), 1024] blocks over mu (= permuted
    m). exp splits per pair: A-tile on Scalar (table exp), B-tile on DVE via
    the Schraudolph int8 bit-trick for pairs < ACT_B (i8 = trunc(S*8/ln2*SCALE
    + B8) bitcast fp8e4).
  - E^T via XBAR DMA transpose (16x128 uint16 tiles) on byte-PAIRED fp8,
    issued from the Activation queue right after the exps it depends on:
    mu is host-permuted so uint16 byte pairs = m-tile pairs, lining up with
    DoubleRow's two k-tiles and the natural [128, 32, 80] v-aug layout.
  - PVs are fp8 DoubleRow (2 k-tiles/instr): o1 streams E natively (contract
    n, ifmap [p, 2, 512]), o2 streams ET byte pairs (contract m, [p, 2, 128]).
    Ones-column in v-aug yields the softmax denominators.
  - o2 partials accumulate into SBUF fp32 (DVE adds), DMA'd out per slice.
  - Outputs ship small: o1T [64, N] bf16 + den1, o2acc [65, N] f32. The final
    64x256 wo projections + divisions run on HOST (2% of FLOPs), like the
    baseline's host-side division/bias handling.
  - Per-block emission interleaves score(b) with PV(b-1) chunk-by-chunk so the
    in-order PE stream fills exp-latency gaps.
Host: permutes k2 columns into mu order, unscrambles o1 rows, projects with
wo, sums the 8 head partials, adds bias constants.
"""

import sys

sys.path.insert(0, "/opt/trn_rl_repo")

import numpy as np

N = 4096
C = 256
D = 64
SCALE = float(D) ** -0.5
NCORES = 8
MBLK = 1024          # mu-block width
NMB = N // MBLK      # 4 blocks
NT = 32              # n-tiles of 128
NQ = NT // 2         # t-pairs per block
ACT_B = 10           # B-side exp of pairs 0..ACT_B-1 goes to DVE
A8 = 11.5415603 * SCALE   # 8/ln2 * SCALE (DVE reads raw psum scores)
B8 = 56.1                 # Schraudolph offset: 55.6 tuned for round-to-nearest
                          # +0.5 because the fp32->int8 convert truncates
                          # (values are all positive, so trunc == floor)
TP_ENGINE = "sync"        # queue for the XBAR transposes: "scalar" | "sync"

_cache: dict = {}


def _build_module():
    import concourse.bacc as bacc
    import concourse.mybir as mybir
    import concourse.tile as tile

    f32 = mybir.dt.float32
    bf16 = mybir.dt.bfloat16
    f8 = mybir.dt.float8e4
    u16 = mybir.dt.uint16
    i8 = mybir.dt.int8
    Exp = mybir.ActivationFunctionType.Exp
    Copy = mybir.ActivationFunctionType.Copy
    DR = mybir.MatmulPerfMode.DoubleRow
    Add = mybir.AluOpType.add
    Mult = mybir.AluOpType.mult

    nc = bacc.Bacc("TRN2", target_bir_lowering=False, debug=False)
    tp_eng = nc.scalar if TP_ENGINE == "scalar" else nc.sync

    def din(name, shape, dt=bf16):
        return nc.dram_tensor(name, shape, dt, kind="ExternalInput").ap()

    k1T = din("k1T", [C, N])      # natural n columns
    k2T = din("k2T", [C, N])      # mu-PERMUTED columns (host)
    v1T = din("v1T", [C, N])
    v2T = din("v2T", [C, N])
    wk1 = din("wk1", [C, 128])    # column-duplicated [wk|wk]
    wk2 = din("wk2", [C, 128])
    wv1 = din("wv1", [C, D])
    wv2 = din("wv2", [C, D])
    bk1 = din("bk1", [128, 1], f32)   # row-duplicated bias
    bk2 = din("bk2", [128, 1], f32)

    o1b = nc.dram_tensor("o1b", [D, N], bf16, kind="ExternalOutput").ap()
    den1 = nc.dram_tensor("den1", [1, N], f32, kind="ExternalOutput").ap()
    o2a = nc.dram_tensor("o2a", [D + 1, N], f32, kind="ExternalOutput").ap()

    with tile.TileContext(nc) as tc:
        with (
            tc.tile_pool(name="const", bufs=1) as constp,
            tc.tile_pool(name="raw", bufs=4) as rawp,
            tc.tile_pool(name="eblk", bufs=2) as ep,
            tc.tile_pool(name="et", bufs=2) as etp,
            tc.tile_pool(name="o1t", bufs=2) as otp,
            tc.tile_pool(name="denp", bufs=2) as denp,
            tc.tile_pool(name="sps", bufs=2, space="PSUM") as sps,
            tc.tile_pool(name="p1", bufs=1, space="PSUM") as p1,
            tc.tile_pool(name="p2", bufs=2, space="PSUM") as p2,
        ):
            # ---- PE warm-up (HAM clock ramp) ----
            warm = constp.tile([128, 512], bf16, tag="warm")
            nc.vector.memset(warm[:], 0.0)
            for _ in range(14):
                wps = p2.tile([128, 512], f32, tag="op")
                nc.tensor.matmul(wps[:], warm[:, 0:128], warm[:], start=True, stop=True)

            # earliest input prefetch: the raws block 0's kproj needs
            pre = []
            for rawT, j in ((k2T, 0), (k2T, 1), (k1T, 0)):
                raw = rawp.tile([128, 2, 512], bf16, tag="raw")
                for ct in range(2):
                    nc.sync.dma_start(
                        out=raw[:, ct, :],
                        in_=rawT[ct * 128:(ct + 1) * 128, j * 512:(j + 1) * 512])
                pre.append(raw)

            # ---- weights ----
            def ld(name, drt, shape):
                t = constp.tile(shape, bf16, tag=name)
                for ct in range(2):
                    nc.sync.dma_start(out=t[:, ct, :], in_=drt[ct * 128:(ct + 1) * 128, :])
                return t

            wk1_sb = ld("wk1", wk1, [128, 2, 128])
            wk2_sb = ld("wk2", wk2, [128, 2, 128])
            wv1_sb = ld("wv1", wv1, [128, 2, D])
            wv2_sb = ld("wv2", wv2, [128, 2, D])
            bk1_sb = constp.tile([128, 1], f32, tag="bk1")
            nc.sync.dma_start(out=bk1_sb[:], in_=bk1[:])
            bk2_sb = constp.tile([128, 1], f32, tag="bk2")
            nc.sync.dma_start(out=bk2_sb[:], in_=bk2[:])

            # o2 accumulator [65, N] fp32 (row 64 = denominator)
            o2acc = constp.tile([D + 1, N], f32, tag="o2acc")
            nc.vector.memset(o2acc[:], 0.0)

            k1p8 = constp.tile([128, N], f8, tag="k1p8")
            k2p8 = constp.tile([128, N], f8, tag="k2p8")
            vaug1 = constp.tile([128, NT, 80], f8, tag="vaug1")
            vaug2 = constp.tile([128, NT, 80], f8, tag="vaug2")
            nc.vector.memset(vaug1[:, :, D:D + 1], 1.0)
            nc.vector.memset(vaug2[:, :, D:D + 1], 1.0)

            # one k-projection chunk -> k?p8[:, 512j:512(j+1)] (d duplicated)
            def k_chunk(rawT, wsb, bsb, dest, j, raw=None):
                if raw is None:
                    raw = rawp.tile([128, 2, 512], bf16, tag="raw")
                    for ct in range(2):
                        nc.sync.dma_start(
                            out=raw[:, ct, :],
                            in_=rawT[ct * 128:(ct + 1) * 128, j * 512:(j + 1) * 512])
                kp = p2.tile([128, 512], f32, tag="op")
                for ct in range(2):
                    nc.tensor.matmul(kp[:], wsb[:, ct, :], raw[:, ct, :],
                                     start=(ct == 0), stop=(ct == 1))
                nc.vector.tensor_scalar(
                    dest[:, j * 512:(j + 1) * 512], kp[:], bsb[:], None, Add)

            # one v-projection chunk -> vaug[:, 4j:4j+4, 0:64]
            def v_chunk(rawT, wsb, dest, j):
                raw = rawp.tile([128, 2, 512], bf16, tag="raw")
                for ct in range(2):
                    nc.sync.dma_start(
                        out=raw[:, ct, :],
                        in_=rawT[ct * 128:(ct + 1) * 128, j * 512:(j + 1) * 512])
                vp = p2.tile([128, 4, D], f32, tag="op")
                for k in range(4):
                    for ct in range(2):
                        nc.tensor.matmul(
                            vp[:, k, :], raw[:, ct, k * 128:(k + 1) * 128],
                            wsb[:, ct, :], start=(ct == 0), stop=(ct == 1))
                nc.vector.tensor_copy(dest[:, 4 * j:4 * j + 4, 0:D], vp[:])

            # minimal prologue: the chunks score block 0 needs first
            k_chunk(k2T, wk2_sb, bk2_sb, k2p8, 0, raw=pre[0])
            k_chunk(k2T, wk2_sb, bk2_sb, k2p8, 1, raw=pre[1])
            k_chunk(k1T, wk1_sb, bk1_sb, k1p8, 0, raw=pre[2])
            # remaining projection work, early in the PE stream; block-0
            # exp-throttling gives PE slack to absorb it
            for j in range(2, 8):
                k_chunk(k2T, wk2_sb, bk2_sb, k2p8, j)
            for j in range(1, 8):
                k_chunk(k1T, wk1_sb, bk1_sb, k1p8, j)
            for j in range(8):
                v_chunk(v1T, wv1_sb, vaug1, j)
            for j in range(8):
                v_chunk(v2T, wv2_sb, vaug2, j)

            state = {}

            def emit_o1_chunk(q, pEhs, o1ps):
                pE = pEhs[q // 8]
                t0 = (2 * q) % (NT // 2)
                for c in range(2):
                    nc.tensor.matmul(
                        o1ps[:, c * 512:(c + 1) * 512],
                        vaug1[:, 2 * q:2 * q + 2, 0:D + 1],
                        pE[:, t0:t0 + 2, c * 512:(c + 1) * 512],
                        perf_mode=DR, start=(q == 0), stop=(q == NQ - 1))

            def emit_o2_chunk(q, pb, pEThs, stream_o2=False):
                n8 = q // 2
                if q % 2 == 0:
                    state[f"o2ps{pb}"] = p2.tile(
                        [D + 1, 512], f32, tag="op", name=f"o2ps_{pb}_{q}")
                o2ps = state[f"o2ps{pb}"]
                pET8 = pEThs[q // 8].bitcast(f8)
                for nl in (0, 1) if q % 2 == 0 else (2, 3):
                    t = (4 * n8 + nl) % (NT // 2)
                    for c2 in range(4):
                        ifm = pET8[:, t, c2, :].rearrange("p (l i) -> p i l", i=2)
                        nc.tensor.matmul(
                            o2ps[:, nl * 128:(nl + 1) * 128],
                            vaug2[:, 8 * pb + 2 * c2:8 * pb + 2 * c2 + 2, 0:D + 1],
                            ifm, perf_mode=DR, start=(c2 == 0), stop=(c2 == 3))
                if q % 2 == 1:
                    nc.vector.tensor_tensor(
                        o2acc[:, n8 * 512:(n8 + 1) * 512],
                        o2ps[:], o2acc[:, n8 * 512:(n8 + 1) * 512], Add)
                    if stream_o2:
                        nc.sync.dma_start(
                            out=o2a[:, n8 * 512:(n8 + 1) * 512],
                            in_=o2acc[:, n8 * 512:(n8 + 1) * 512])

            def emit_pv_chunk(q, pb, pEhs, pEThs, o1ps, stream_o2=False):
                emit_o1_chunk(q, pEhs, o1ps)
                emit_o2_chunk(q, pb, pEThs, stream_o2)

            def emit_o1_epilogue(pb, o1ps):
                o1t = otp.tile([D, MBLK], bf16, tag="o1t")
                d1 = denp.tile([1, MBLK], f32, tag="d1")
                nc.scalar.activation(o1t[:], o1ps[0:D, :], Copy)
                nc.vector.tensor_copy(d1[:], o1ps[D:D + 1, :])
                nc.sync.dma_start(out=o1b[:, pb * MBLK:(pb + 1) * MBLK], in_=o1t[:])
                nc.sync.dma_start(out=den1[0:1, pb * MBLK:(pb + 1) * MBLK], in_=d1[:])

            def emit_block(mb):
                """score+exp+transpose for block mb, interleaved per t-pair
                with the PV of the previous block."""
                Ehs, EThs = [], []
                prev = state.get("prev")
                if prev is not None:
                    pb, pEhs, pEThs = prev
                    o1ps = p1.tile([D + 1, MBLK], f32, tag="o1",
                                   name=f"o1ps_{mb}")
                for q in range(NQ):
                    if q % 8 == 0:
                        Ehs.append(ep.tile([128, NT // 2, MBLK], f8,
                                           tag=f"eblk{q // 8}",
                                           name=f"E_{mb}_{q // 8}"))
                        EThs.append(etp.tile([128, NT // 2, 4, 128], u16,
                                             tag=f"et{q // 8}",
                                             name=f"ET_{mb}_{q // 8}"))
                    E, ET = Ehs[q // 8], EThs[q // 8]
                    E8i = E.bitcast(i8)
                    Eu16 = E.bitcast(u16)
                    t0, t1 = (2 * q) % NT, (2 * q + 1) % NT
                    spA = sps.tile([128, MBLK], f32, tag="sp")
                    spB = sps.tile([128, MBLK], f32, tag="sp")
                    for c in range(2):
                        lo = mb * MBLK + c * 512
                        nc.tensor.matmul(
                            spA[:, c * 512:(c + 1) * 512],
                            k1p8[0:64, t0 * 128:(t0 + 1) * 128],
                            k2p8[0:64, lo:lo + 512], start=True, stop=True)
                        nc.tensor.matmul(
                            spB[:, c * 512:(c + 1) * 512],
                            k1p8[64:128, t1 * 128:(t1 + 1) * 128],
                            k2p8[64:128, lo:lo + 512], start=True, stop=True)
                    tl0 = (2 * q) % (NT // 2)
                    act_b = 6 if mb == 0 else ACT_B
                    nc.scalar.activation(E[:, tl0, :], spA[:], Exp, scale=SCALE)
                    if q < act_b:
                        nc.vector.tensor_scalar(
                            E8i[:, tl0 + 1, :], spB[:], A8, B8, Mult, Add)
                    else:
                        nc.scalar.activation(E[:, tl0 + 1, :], spB[:], Exp, scale=SCALE)
                    if q % 2 == 1:
                        tp_eng.dma_start_transpose(
                            ET[:, tl0 - 2:tl0 + 2], Eu16[:, tl0 - 2:tl0 + 2])
                    if prev is not None:
                        emit_pv_chunk(q, pb, pEhs, pEThs, o1ps)
                    if mb == NMB - 1 and q >= 2:
                        # interleave the last block's own o2-PV at lag 2
                        emit_o2_chunk(q - 2, mb, EThs, stream_o2=True)
                if prev is not None:
                    emit_o1_epilogue(pb, o1ps)
                state["prev"] = (mb, Ehs, EThs)

            for mb in range(NMB):
                emit_block(mb)
            # tail: the last block's o1-PV + final o2 chunks
            pb, pEhs, pEThs = state["prev"]
            o1ps = p1.tile([D + 1, MBLK], f32, tag="o1", name="o1ps_tail")
            for q in range(NQ):
                emit_o1_chunk(q, pEhs, o1ps)
                if q in (NQ - 2, NQ - 1):
                    emit_o2_chunk(q, pb, pEThs, stream_o2=True)
            emit_o1_epilogue(pb, o1ps)

    nc.compile()
    return nc


def _mu_perm():
    """mu -> m mapping: mu = 1024*mb + 256*c2 + 2*q + i  ->
    m = 1024*mb + 128*(2*c2 + i) + q."""
    mu = np.arange(N)
    mb = mu // MBLK
    r = mu % MBLK
    c2 = r // 256
    q = (r % 256) // 2
    i = mu % 2
    return (MBLK * mb + 128 * (2 * c2 + i) + q).astype(np.int64)


def _get_nc():
    if "nc" not in _cache:
        _cache["nc"] = _build_module()
    return _cache["nc"]


def kernel(k1, v1, k2, v2,
           wk1_w, wk1_b, wv1_w, wv1_b,
           wk2_w, wk2_b, wv2_w, wv2_b,
           wo1_w, wo1_b, wo2_w, wo2_b):
    import ml_dtypes
    from concourse.bass_utils import run_bass_kernel_spmd

    nc = _get_nc()

    f = np.float32
    bf = ml_dtypes.bfloat16
    perm = _mu_perm()

    k1T = np.ascontiguousarray(np.asarray(k1, f).T).astype(bf)
    v1T = np.ascontiguousarray(np.asarray(v1, f).T).astype(bf)
    k2T = np.ascontiguousarray(np.asarray(k2, f).T[:, perm]).astype(bf)
    v2T = np.ascontiguousarray(np.asarray(v2, f).T).astype(bf)

    def dup2(a):   # [C, 64] -> [C, 128] column-duplicated
        return np.ascontiguousarray(np.concatenate([a, a], axis=1))

    in_maps = []
    for h in range(NCORES):
        sl = slice(h * D, (h + 1) * D)
        in_maps.append({
            "k1T": k1T, "v1T": v1T, "k2T": k2T, "v2T": v2T,
            "wk1": dup2(np.asarray(wk1_w, f)[:, sl]).astype(bf),
            "wk2": dup2(np.asarray(wk2_w, f)[:, sl]).astype(bf),
            "wv1": np.ascontiguousarray(np.asarray(wv1_w, f)[:, sl]).astype(bf),
            "wv2": np.ascontiguousarray(np.asarray(wv2_w, f)[:, sl]).astype(bf),
            "bk1": np.ascontiguousarray(
                np.tile(np.asarray(wk1_b, f)[sl].reshape(D, 1), (2, 1))),
            "bk2": np.ascontiguousarray(
                np.tile(np.asarray(wk2_b, f)[sl].reshape(D, 1), (2, 1))),
        })

    res = run_bass_kernel_spmd(nc, in_maps, list(range(NCORES)))
    _cache["last_result"] = res

    out1 = np.zeros((N, C), f)
    out2 = np.zeros((N, C), f)
    for h in range(NCORES):
        sl = slice(h * D, (h + 1) * D)
        rh = res.results[h]
        t1 = (np.asarray(rh["o1b"], f) / rh["den1"]).T @ np.asarray(wo1_w, f)[sl, :]
        out1[perm] += t1                     # unscramble mu -> m
        o2acc = np.asarray(rh["o2a"], f)
        out2 += (o2acc[0:D, :] / o2acc[D:D + 1, :]).T @ np.asarray(wo2_w, f)[sl, :]
    out1 += np.asarray(wv1_b, f) @ np.asarray(wo1_w, f) + np.asarray(wo1_b, f)
    out2 += np.asarray(wv2_b, f) @ np.asarray(wo2_w, f) + np.asarray(wo2_b, f)
    return out1, out2
